# revision 25
# baseline (speedup 1.0000x reference)
"""CARAFE kernel for Trainium2 (8 NeuronCores, batch-parallel), v3.

Reference computation per image (one per core):
  R = relu(conv1x1(x, w_compress, b_compress))          [48, 128, 128]
  E = conv3x3(R, w_encoder, b_encoder, pad=1)           [100, 128, 128]
  Y = softmax over k of E.reshape(4, 25, H, W)          (s, k, h, w)
  out[s,c,h,w] = sum_k Y[s,k,h,w] * xpad[c, h+dy, w+dx] (k=(dy,dx), 5x5, pad 2)
  pixel-shuffle: out_ref[s*16 + c//4, 2h + (c//2)%2, 2w + c%2] = out[s,c,h,w]

Mapping (all 16-bit datapaths; measured E range is [-3.2, 3.3] so fp16
holds exp(E) and every intermediate comfortably):
  - conv1x1: fp16 matmuls, bias+relu fused in the PSUM->SBUF activation
    (bias as a per-partition vector).
  - conv3x3 over a zero-padded 130x130 R grid, 6 matmuls per 512-pixel
    chunk instead of 9: the R grid holds a second copy of R shifted by one
    element (rows 48-95, built by one SBUF->SBUF DMA), so taps (ty,0) and
    (ty,1) share a matmul with a 97-row contraction.  Bias via a ones row.
    exp fused into the PSUM->SBUF copy on ScalarE.
  - F^T transpose and the softmax denominator in ONE matmul per w-column:
    the moving operand is [I_100 | S] where S sums each s-group of 25, so
    PSUM gets F^T columns and Z^T columns together.  PSUM->SBUF epilogue
    copies run on VectorE (idle during the prefix); ScalarE keeps
    relu/exp/X^T epilogues.
  - X^T runs on the PE after conv3x3 (off the critical path to the
    normalize).  Its PSUM epilogue writes BOTH parity copies (XE0 and
    XO_0) so the dy=0 taps never wait on a DMA.
  - patch sum on VectorE in fp16 (2x perf mode): pixel-major layout
    [128 h-partitions, (c, w)].  dx taps are free-dim offsets; odd dx
    reads a one-element-shifted copy so every operand stays 4B-aligned.
    dy taps read partition-shifted copies of XE0 built by contiguous
    full-row SBUF->SBUF DMA into a 3-slot ring (+ the XO_0 tile), all
    prefetched one dy-phase ahead; edge partitions are zeroed (halo DMA).
  - pixel shuffle via a strided ScalarE copy (fp16->fp32) into
    (c4, r1, w, r2) order, then DMA with 2KB-contiguous runs.
"""

import sys

import numpy as np

sys.path.insert(0, "/opt/trn_rl_repo")

import concourse.bass as bass
import concourse.mybir as mybir
import concourse.tile as tile
from concourse import bacc

F32 = mybir.dt.float32
F16 = mybir.dt.float16

H = 128
W = 128
C = 64
M = 48  # compressed channels
S2 = 4  # scale_factor**2
K2 = 25  # k_up**2
SK = 100
HW = H * W
WP = 132  # padded row width in pixel-major x buffers
XF = C * WP  # 8448 free elems per partition
G = 130 * 130  # padded R grid
N_CORES = 8

# conv3x3 tap blocks: (moving ty, moving tx); rows 0-47 of the stationary
# hold tap (ty,tx), rows 48-95 hold tap (ty,tx+1) via the shifted R copy,
# row 96 rides the ones row (bias on block 1 only).
BLOCKS = [(0, 0, True), (1, 0, True), (2, 0, True), (0, 2, False), (1, 2, False), (2, 2, False)]


def _ap(t, extra_off, dims):
    """Raw AP on a tile handle `t` with free-offset `extra_off` (elements)
    and explicit [step, count] dims (dims[0] is the partition dim)."""
    base = t[:]
    return bass.AP(tensor=base.tensor, offset=base.offset + extra_off, ap=dims)


class _Pool:
    """Manually scoped tile pool."""

    def __init__(self, tc, **kw):
        self._cm = tc.tile_pool(**kw)
        self.pool = self._cm.__enter__()
        self._n = 0

    def tile(self, *a, tag=None, **kw):
        self._n += 1
        t = tag or f"t{self._n}"
        return self.pool.tile(*a, tag=t, name=t, **kw)

    def close(self):
        self._cm.__exit__(None, None, None)


def build_program():
    nc = bacc.Bacc("TRN2", target_bir_lowering=False, debug=False)

    xin = nc.dram_tensor("xin", [C, HW], F32, kind="ExternalInput")
    w1te = nc.dram_tensor("w1te", [C, M], F16, kind="ExternalInput")
    b1 = nc.dram_tensor("b1", [M, 1], F32, kind="ExternalInput")
    wete = nc.dram_tensor("wete", [113, 6 * SK], F16, kind="ExternalInput")
    identc = nc.dram_tensor("identc", [C, C], F16, kind="ExternalInput")
    idents = nc.dram_tensor("idents", [SK, SK + S2], F16, kind="ExternalInput")
    ones16 = nc.dram_tensor("ones16", [1, G], F16, kind="ExternalInput")
    zer16 = nc.dram_tensor("zer16", [2, XF], F16, kind="ExternalInput")
    out = nc.dram_tensor("out", [C, 4 * HW], F32, kind="ExternalOutput")
    x16d = nc.dram_tensor("x16d", [C, HW], F16, kind="Internal")

    with tile.TileContext(nc) as tc:
        cp = _Pool(tc, name="consts", bufs=1)
        w1te_sb = cp.tile([C, M], F16, tag="w1te")
        nc.sync.dma_start(w1te_sb[:], w1te.ap())
        b1_sb = cp.tile([M, 1], F32, tag="b1")
        nc.sync.dma_start(b1_sb[:], b1.ap())
        wete_sb = cp.tile([113, 6 * SK], F16, tag="wete")
        nc.sync.dma_start(wete_sb[:], wete.ap())
        identc_sb = cp.tile([C, C], F16, tag="identc")
        nc.sync.dma_start(identc_sb[:], identc.ap())
        idents_sb = cp.tile([SK, SK + S2], F16, tag="idents")
        nc.sync.dma_start(idents_sb[:], idents.ap())

        # persistent through the patch-sum phase
        pxe0 = _Pool(tc, name="pxe0", bufs=1)
        XE0 = pxe0.tile([128, XF], F16, tag="xe0")
        pfr = _Pool(tc, name="pfr", bufs=1)
        fr = pfr.tile([128, SK * W], F16, tag="fr")
        prz = _Pool(tc, name="prz", bufs=1)
        rz = prz.tile([128, S2 * W], F16, tag="rz")
        pxo0 = _Pool(tc, name="pxo0", bufs=1)
        XO0 = pxo0.tile([128, XF], F16, tag="xo0")
        # w-halo columns of the pixel-major x buffers (before the DMA builds)
        nc.vector.memset(_ap(XE0, 0, [[XF, 128], [WP, C], [1, 2]]), 0.0)
        nc.vector.memset(_ap(XE0, 130, [[XF, 128], [WP, C], [1, 2]]), 0.0)
        nc.vector.memset(_ap(XO0, 0, [[XF, 128], [WP, C], [1, 1]]), 0.0)
        nc.vector.memset(_ap(XO0, 129, [[XF, 128], [WP, C], [1, 3]]), 0.0)

        # ---- load x first (cast fp32->fp16 during DMA, 4 parallel engines) ----
        pxa = _Pool(tc, name="pxa", bufs=1)
        x16 = pxa.tile([C, HW], F16, tag="x16")
        for c in range(0, C, 8):
            nc.gpsimd.dma_start(x16[c : c + 8, :], xin.ap()[c : c + 8, :])

        # stage x16 to DRAM, then build the pixel-major XE0/XO0 buffers with
        # strided DMAs (bypasses the PE transpose entirely; runs on otherwise
        # idle DMA engines in parallel with the conv chain)
        for i in range(4):
            c0, c1 = C * i // 4, C * (i + 1) // 4
            eng = nc.sync if i % 2 == 0 else nc.scalar
            eng.dma_start(x16d.ap()[c0:c1, :], x16[c0:c1, :])
        for i in range(8):
            p0, p1 = 128 * i // 8, 128 * (i + 1) // 8
            eng = nc.sync if i % 2 == 0 else nc.scalar
            eng.dma_start(
                _ap(XE0, p0 * XF + 2, [[XF, p1 - p0], [WP, C], [1, W]]),
                bass.AP(tensor=x16d, offset=p0 * W,
                        ap=[[W, p1 - p0], [HW, C], [1, W]]),
            )
            eng2 = nc.scalar if i % 2 == 0 else nc.sync
            eng2.dma_start(
                _ap(XO0, p0 * XF + 1, [[XF, p1 - p0], [WP, C], [1, W]]),
                bass.AP(tensor=x16d, offset=p0 * W,
                        ap=[[W, p1 - p0], [HW, C], [1, W]]),
            )

        pf = _Pool(tc, name="pf", bufs=1)
        F = pf.tile([SK, HW], F16, tag="F")

        # ---- R grid (with shifted duplicate rows 48-95, ones row 96) ----
        pr = _Pool(tc, name="pr", bufs=1)
        R = pr.tile([113, G], F16, tag="R")
        # zero only the padding cells (grid rows 0/129, cols 0/129 for rows
        # 0-47; the shifted duplicate rows inherit them via the dup copy)
        nc.vector.memset(_ap(R, 0, [[G, M], [1, 130]]), 0.0)
        nc.vector.memset(_ap(R, 129 * 130, [[G, M], [1, 130]]), 0.0)
        nc.vector.memset(_ap(R, 130, [[G, M], [130, 128], [1, 1]]), 0.0)
        nc.vector.memset(_ap(R, 130 + 129, [[G, M], [130, 128], [1, 1]]), 0.0)
        nc.vector.memset(_ap(R, 32 * G, [[G, 32], [1, G]]), 0.0)
        nc.vector.memset(_ap(R, 64 * G, [[G, 48], [1, 130]]), 0.0)
        nc.vector.memset(_ap(R, 64 * G + 129 * 130, [[G, 48], [1, 130]]), 0.0)
        nc.vector.memset(_ap(R, 64 * G + 128, [[G, 48], [130, 130], [1, 2]]), 0.0)
        nc.sync.dma_start(_ap(R, 112 * G, [[G, 1], [1, G]]), ones16.ap())

        # ---- conv1x1 + bias + relu into R interior ----
        psA = _Pool(tc, name="psA", bufs=4, space="PSUM")
        for j in range(32):
            ps1 = psA.tile([M, 512], F32, tag="ps1")
            nc.tensor.matmul(
                ps1[:], w1te_sb[:], x16[:, j * 512 : (j + 1) * 512],
                start=True, stop=True,
            )
            for base, coff in ((0, 1), (64 * G, 0)):
                dst = _ap(R, base + (4 * j + 1) * 130 + coff, [[G, M], [130, 4], [1, W]])
                if (2 * j + (base != 0)) % 3 < 2:
                    nc.vector.tensor_scalar(
                        dst, ps1[:], b1_sb[:], 0.0,
                        mybir.AluOpType.add, mybir.AluOpType.max,
                    )
                else:
                    nc.scalar.activation(
                        dst, ps1[:], mybir.ActivationFunctionType.Relu,
                        bias=b1_sb[:],
                    )
        psA.close()

        # ---- conv3x3 (6 paired blocks) + exp -> F ----
        psB = _Pool(tc, name="psB", bufs=4, space="PSUM")
        for j in range(32):
            ps2 = psB.tile([SK, 512], F32, tag="ps2")
            for b, (ty, tx, _pair) in enumerate(BLOCKS):
                nc.tensor.matmul(
                    ps2[:],
                    wete_sb[:, b * SK : (b + 1) * SK],
                    _ap(R, (4 * j + ty) * 130 + tx, [[G, 113], [130, 4], [1, W]]),
                    start=(b == 0), stop=(b == len(BLOCKS) - 1),
                )
            nc.scalar.activation(
                F[:, j * 512 : (j + 1) * 512], ps2[:],
                mybir.ActivationFunctionType.Exp,
            )
        psB.close()
        pr.close()

        # ---- F^T + Z in one matmul per w-column (4 per PSUM bank) ----
        SZ = SK + S2
        psF = _Pool(tc, name="psF", bufs=4, space="PSUM")
        for wb in range(32):
            pst = psF.tile([128, 4 * SZ], F32, tag="pstf")
            for w4 in range(4):
                w = wb * 4 + w4
                nc.tensor.matmul(
                    pst[:, w4 * SZ : (w4 + 1) * SZ],
                    _ap(F, w, [[HW, SK], [W, H]]),
                    idents_sb[:],
                    start=True, stop=True,
                )
            nc.vector.tensor_copy(
                _ap(fr, wb * 4, [[SK * W, 128], [W, SK], [1, 4]]),
                _ap(pst, 0, [[4 * SZ, 128], [1, SK], [SZ, 4]]),
            )
            nc.vector.tensor_copy(
                _ap(rz, wb * 4, [[S2 * W, 128], [W, S2], [1, 4]]),
                _ap(pst, SK, [[4 * SZ, 128], [1, S2], [SZ, 4]]),
            )
        psF.close()
        pf.close()

        pxa.close()

        # ---- softmax normalize: fr *= 1/Z (broadcast over k) ----
        with nc.allow_low_precision(reason="softmax weights tolerate fp16 recip"):
            nc.vector.reciprocal(rz[:], rz[:])
        fr_bc = _ap(fr, 0, [[SK * W, 128], [K2 * W, S2], [W, K2], [1, W]])
        nc.vector.tensor_mul(
            fr_bc,
            fr_bc,
            _ap(rz, 0, [[S2 * W, 128], [W, S2], [0, K2], [1, W]]),
        )

        # ---- patch-sum pools: 3-slot ring + XO0 for shifted x copies ----
        slotP = _Pool(tc, name="slots", bufs=1)
        slots = [slotP.tile([128, XF], F16, tag=f"sl{i}") for i in range(3)]
        accP = _Pool(tc, name="acc", bufs=1)
        accs = [accP.tile([128, C * W], F16, tag=f"a{s}") for s in range(S2)]
        tmpP = _Pool(tc, name="tmp", bufs=1)
        tmp = tmpP.tile([128, C * W], F16, tag="tmp")
        acc2P = _Pool(tc, name="acc2", bufs=2)

        # slot assignment per dy phase (current pair + prefetch pair alternate)
        slot_for = {-1: (slots[0], slots[1]), 1: (slots[2], XO0),
                    -2: (slots[0], slots[1]), 2: (slots[2], XO0)}
        bufs = {0: (XE0, XO0)}

        def issue_copies(dy):
            p0, p1 = max(0, -dy), 128 - max(0, dy)
            XE, XO = slot_for[dy]
            # split each body copy into 4 partition chunks on alternating
            # HWDGE queues: one dma_start lands on ONE SDMA engine (~27GB/s),
            # so chunking is what buys DMA parallelism.
            for X, eoff in ((XE, 0), (XO, 1)):
                bounds = [p0 + (p1 - p0) * i // 4 for i in range(5)]
                for c0, c1 in zip(bounds, bounds[1:]):
                    eng = nc.sync if (eoff + c0) % 2 == 0 else nc.scalar
                    eng.dma_start(
                        _ap(X, c0 * XF, [[XF, c1 - c0], [1, XF - eoff]]),
                        _ap(XE0, (c0 + dy) * XF + eoff, [[XF, c1 - c0], [1, XF - eoff]]),
                    )
            nh = abs(dy)
            p0h = 0 if dy < 0 else 128 - dy
            for X in (XE, XO):
                nc.sync.dma_start(
                    _ap(X, p0h * XF, [[XF, nh], [1, XF]]), zer16.ap()[0:nh, :]
                )
            bufs[dy] = (XE, XO)

        # ---- pixel shuffle + store: per (s, eighth) ----
        def store_s(s):
            for q in range(8):
                acc2 = acc2P.tile([128, 1024], F32, tag="acc2")
                # iteration (c4', r1, w, r2); src channel c = 8q + 4c4' + 2r1 + r2
                nc.scalar.copy(
                    _ap(acc2, 0, [[1024, 128], [512, 2], [256, 2], [1, 2], [2, W]]),
                    _ap(
                        accs[s],
                        8 * q * W,
                        [[C * W, 128], [4 * W, 2], [2 * W, 2], [W, 2], [1, W]],
                    ),
                )
                for c4 in range(2):
                    for hh in range(2):
                        dst = bass.AP(
                            tensor=out,
                            offset=(s * 16 + 2 * q + c4) * 4 * HW + hh * 64 * 512,
                            ap=[[2 * 2 * W, 64], [1, 512]],
                        )
                        eng = nc.sync if (q + c4 + hh) % 2 == 0 else nc.scalar
                        eng.dma_start(
                            dst,
                            _ap(acc2, hh * 64 * 1024 + c4 * 512, [[1024, 64], [1, 512]]),
                        )

        # ---- patch sum on VectorE ----
        DY_ORDER = [0, -1, 1, -2, 2]
        DX_ORDER = [-2, 0, 2, -1, 1]

        for di, dy in enumerate(DY_ORDER):
            if di + 1 < len(DY_ORDER):
                issue_copies(DY_ORDER[di + 1])
            XE, XO = bufs.pop(dy)
            for s in range(S2):
                for dx in DX_ORDER:
                    k = (dy + 2) * 5 + (dx + 2)
                    if dx % 2 == 0:
                        src, off = XE, 2 + dx
                    else:
                        src, off = XO, 1 + dx
                    in0 = _ap(src, off, [[XF, 128], [WP, C], [1, W]])
                    in1 = _ap(
                        fr, (s * K2 + k) * W, [[SK * W, 128], [0, C], [1, W]]
                    )
                    if dy == 0 and dx == DX_ORDER[0]:
                        dst = _ap(accs[s], 0, [[C * W, 128], [W, C], [1, W]])
                        nc.vector.tensor_mul(dst, in0, in1)
                    else:
                        dstt = _ap(tmp, 0, [[C * W, 128], [W, C], [1, W]])
                        nc.vector.tensor_mul(dstt, in0, in1)
                        nc.vector.tensor_add(accs[s][:], accs[s][:], tmp[:])
                if dy == DY_ORDER[-1]:
                    store_s(s)

        acc2P.close()
        tmpP.close()
        accP.close()
        slotP.close()
        pxo0.close()
        prz.close()
        pfr.close()
        pxe0.close()
        cp.close()
    nc.compile()
    return nc


def host_inputs(x_img, w_compress, b_compress, w_encoder, b_encoder):
    """Per-core input map for one image [C, H, W]."""
    w1te = w_compress[:, :, 0, 0].T.astype(np.float16)
    wete = np.zeros((113, 6, SK), np.float16)
    for b, (ty, tx, pair) in enumerate(BLOCKS):
        wete[:M, b, :] = w_encoder[:, :, ty, tx].T
        if pair:
            wete[64 : 64 + M, b, :] = w_encoder[:, :, ty, tx + 1].T
    wete[112, 1, :] = b_encoder
    identc = np.eye(C, dtype=np.float16)
    idents = np.zeros((SK, SK + S2), np.float16)
    idents[:, :SK] = np.eye(SK)
    for s in range(S2):
        idents[s * K2 : (s + 1) * K2, SK + s] = 1.0
    return {
        "xin": np.ascontiguousarray(x_img.reshape(C, HW)).astype(np.float32),
        "w1te": w1te,
        "b1": b_compress.reshape(M, 1).astype(np.float32),
        "wete": wete.reshape(113, 6 * SK),
        "identc": identc,
        "idents": idents,
        "ones16": np.ones((1, G), np.float16),
        "zer16": np.zeros((2, XF), np.float16),
    }


_CACHE = {}


def kernel(x, w_compress, b_compress, w_encoder, b_encoder):
    x = np.asarray(x, np.float32)
    if "nc" not in _CACHE:
        _CACHE["nc"] = build_program()
    nc = _CACHE["nc"]
    in_maps = [
        host_inputs(
            x[i],
            np.asarray(w_compress, np.float32),
            np.asarray(b_compress, np.float32),
            np.asarray(w_encoder, np.float32),
            np.asarray(b_encoder, np.float32),
        )
        for i in range(N_CORES)
    ]
    from concourse.bass_utils import run_bass_kernel_spmd

    res = run_bass_kernel_spmd(nc, in_maps, core_ids=list(range(N_CORES)))
    return np.stack(
        [res.results[i]["out"].reshape(C, 2 * H, 2 * W) for i in range(N_CORES)],
        axis=0,
    )


# revision 28
# speedup vs baseline: 1.0034x; 1.0034x over previous
"""CARAFE kernel for Trainium2 (8 NeuronCores, batch-parallel), v3.

Reference computation per image (one per core):
  R = relu(conv1x1(x, w_compress, b_compress))          [48, 128, 128]
  E = conv3x3(R, w_encoder, b_encoder, pad=1)           [100, 128, 128]
  Y = softmax over k of E.reshape(4, 25, H, W)          (s, k, h, w)
  out[s,c,h,w] = sum_k Y[s,k,h,w] * xpad[c, h+dy, w+dx] (k=(dy,dx), 5x5, pad 2)
  pixel-shuffle: out_ref[s*16 + c//4, 2h + (c//2)%2, 2w + c%2] = out[s,c,h,w]

Mapping (all 16-bit datapaths; measured E range is [-3.2, 3.3] so fp16
holds exp(E) and every intermediate comfortably):
  - conv1x1: fp16 matmuls, bias+relu fused in the PSUM->SBUF activation
    (bias as a per-partition vector).
  - conv3x3 over a zero-padded 130x130 R grid, 6 matmuls per 512-pixel
    chunk instead of 9: the R grid holds a second copy of R shifted by one
    element (rows 48-95, built by one SBUF->SBUF DMA), so taps (ty,0) and
    (ty,1) share a matmul with a 97-row contraction.  Bias via a ones row.
    exp fused into the PSUM->SBUF copy on ScalarE.
  - F^T transpose and the softmax denominator in ONE matmul per w-column:
    the moving operand is [I_100 | S] where S sums each s-group of 25, so
    PSUM gets F^T columns and Z^T columns together.  PSUM->SBUF epilogue
    copies run on VectorE (idle during the prefix); ScalarE keeps
    relu/exp/X^T epilogues.
  - X^T runs on the PE after conv3x3 (off the critical path to the
    normalize).  Its PSUM epilogue writes BOTH parity copies (XE0 and
    XO_0) so the dy=0 taps never wait on a DMA.
  - patch sum on VectorE in fp16 (2x perf mode): pixel-major layout
    [128 h-partitions, (c, w)].  dx taps are free-dim offsets; odd dx
    reads a one-element-shifted copy so every operand stays 4B-aligned.
    dy taps read partition-shifted copies of XE0 built by contiguous
    full-row SBUF->SBUF DMA into a 3-slot ring (+ the XO_0 tile), all
    prefetched one dy-phase ahead; edge partitions are zeroed (halo DMA).
  - pixel shuffle via a strided ScalarE copy (fp16->fp32) into
    (c4, r1, w, r2) order, then DMA with 2KB-contiguous runs.
"""

import sys

import numpy as np

sys.path.insert(0, "/opt/trn_rl_repo")

import concourse.bass as bass
import concourse.mybir as mybir
import concourse.tile as tile
from concourse import bacc

F32 = mybir.dt.float32
F16 = mybir.dt.float16

H = 128
W = 128
C = 64
M = 48  # compressed channels
S2 = 4  # scale_factor**2
K2 = 25  # k_up**2
SK = 100
HW = H * W
WP = 132  # padded row width in pixel-major x buffers
XF = C * WP  # 8448 free elems per partition
G = 130 * 130  # padded R grid
N_CORES = 8

# conv3x3 tap blocks: (moving ty, moving tx); rows 0-47 of the stationary
# hold tap (ty,tx), rows 48-95 hold tap (ty,tx+1) via the shifted R copy,
# row 96 rides the ones row (bias on block 1 only).
BLOCKS = [(0, 0, True), (1, 0, True), (2, 0, True), (0, 2, False), (1, 2, False), (2, 2, False)]


def _ap(t, extra_off, dims):
    """Raw AP on a tile handle `t` with free-offset `extra_off` (elements)
    and explicit [step, count] dims (dims[0] is the partition dim)."""
    base = t[:]
    return bass.AP(tensor=base.tensor, offset=base.offset + extra_off, ap=dims)


class _Pool:
    """Manually scoped tile pool."""

    def __init__(self, tc, **kw):
        self._cm = tc.tile_pool(**kw)
        self.pool = self._cm.__enter__()
        self._n = 0

    def tile(self, *a, tag=None, **kw):
        self._n += 1
        t = tag or f"t{self._n}"
        return self.pool.tile(*a, tag=t, name=t, **kw)

    def close(self):
        self._cm.__exit__(None, None, None)


def build_program():
    nc = bacc.Bacc("TRN2", target_bir_lowering=False, debug=False)

    xin = nc.dram_tensor("xin", [C, HW], F32, kind="ExternalInput")
    w1te = nc.dram_tensor("w1te", [C, M], F16, kind="ExternalInput")
    b1 = nc.dram_tensor("b1", [M, 1], F32, kind="ExternalInput")
    wete = nc.dram_tensor("wete", [113, 6 * SK], F16, kind="ExternalInput")
    identc = nc.dram_tensor("identc", [C, C], F16, kind="ExternalInput")
    idents = nc.dram_tensor("idents", [SK, SK + S2], F16, kind="ExternalInput")
    ones16 = nc.dram_tensor("ones16", [1, G], F16, kind="ExternalInput")
    zer16 = nc.dram_tensor("zer16", [2, XF], F16, kind="ExternalInput")
    out = nc.dram_tensor("out", [C, 4 * HW], F32, kind="ExternalOutput")
    x16d = nc.dram_tensor("x16d", [C, HW], F16, kind="Internal")

    with tile.TileContext(nc) as tc:
        cp = _Pool(tc, name="consts", bufs=1)
        w1te_sb = cp.tile([C, M], F16, tag="w1te")
        nc.sync.dma_start(w1te_sb[:], w1te.ap())
        b1_sb = cp.tile([M, 1], F32, tag="b1")
        nc.sync.dma_start(b1_sb[:], b1.ap())
        wete_sb = cp.tile([113, 6 * SK], F16, tag="wete")
        nc.sync.dma_start(wete_sb[:], wete.ap())
        identc_sb = cp.tile([C, C], F16, tag="identc")
        nc.sync.dma_start(identc_sb[:], identc.ap())
        idents_sb = cp.tile([SK, SK + S2], F16, tag="idents")
        nc.sync.dma_start(idents_sb[:], idents.ap())

        # persistent through the patch-sum phase
        pxe0 = _Pool(tc, name="pxe0", bufs=1)
        XE0 = pxe0.tile([128, XF], F16, tag="xe0")
        pfr = _Pool(tc, name="pfr", bufs=1)
        fr = pfr.tile([128, SK * W], F16, tag="fr")
        prz = _Pool(tc, name="prz", bufs=1)
        rz = prz.tile([128, S2 * W], F16, tag="rz")
        pxo0 = _Pool(tc, name="pxo0", bufs=1)
        XO0 = pxo0.tile([128, XF], F16, tag="xo0")
        # w-halo columns of the pixel-major x buffers (before the DMA builds)
        nc.vector.memset(_ap(XE0, 0, [[XF, 128], [WP, C], [1, 2]]), 0.0)
        nc.vector.memset(_ap(XE0, 130, [[XF, 128], [WP, C], [1, 2]]), 0.0)
        nc.vector.memset(_ap(XO0, 0, [[XF, 128], [WP, C], [1, 1]]), 0.0)
        nc.vector.memset(_ap(XO0, 129, [[XF, 128], [WP, C], [1, 3]]), 0.0)

        # ---- load x first (cast fp32->fp16 during DMA, 4 parallel engines) ----
        pxa = _Pool(tc, name="pxa", bufs=1)
        x16 = pxa.tile([C, HW], F16, tag="x16")
        for c in range(0, C, 8):
            nc.gpsimd.dma_start(x16[c : c + 8, :], xin.ap()[c : c + 8, :])

        # stage x16 to DRAM, then build the pixel-major XE0/XO0 buffers with
        # strided DMAs (bypasses the PE transpose entirely; runs on otherwise
        # idle DMA engines in parallel with the conv chain)
        for i in range(4):
            c0, c1 = C * i // 4, C * (i + 1) // 4
            eng = nc.sync if i % 2 == 0 else nc.scalar
            eng.dma_start(x16d.ap()[c0:c1, :], x16[c0:c1, :])
        for i in range(8):
            p0, p1 = 128 * i // 8, 128 * (i + 1) // 8
            eng = nc.sync if i % 2 == 0 else nc.scalar
            eng.dma_start(
                _ap(XE0, p0 * XF + 2, [[XF, p1 - p0], [WP, C], [1, W]]),
                bass.AP(tensor=x16d, offset=p0 * W,
                        ap=[[W, p1 - p0], [HW, C], [1, W]]),
            )
            eng2 = nc.scalar if i % 2 == 0 else nc.sync
            eng2.dma_start(
                _ap(XO0, p0 * XF + 1, [[XF, p1 - p0], [WP, C], [1, W]]),
                bass.AP(tensor=x16d, offset=p0 * W,
                        ap=[[W, p1 - p0], [HW, C], [1, W]]),
            )

        pf = _Pool(tc, name="pf", bufs=1)
        F = pf.tile([SK, HW], F16, tag="F")

        # ---- R grid (with shifted duplicate rows 48-95, ones row 96) ----
        pr = _Pool(tc, name="pr", bufs=1)
        R = pr.tile([113, G], F16, tag="R")
        # zero only the padding cells (grid rows 0/129, cols 0/129 for rows
        # 0-47; the shifted duplicate rows inherit them via the dup copy)
        nc.vector.memset(_ap(R, 0, [[G, M], [1, 130]]), 0.0)
        nc.vector.memset(_ap(R, 129 * 130, [[G, M], [1, 130]]), 0.0)
        nc.vector.memset(_ap(R, 130, [[G, M], [130, 128], [1, 1]]), 0.0)
        nc.vector.memset(_ap(R, 130 + 129, [[G, M], [130, 128], [1, 1]]), 0.0)
        nc.vector.memset(_ap(R, 32 * G, [[G, 32], [1, G]]), 0.0)
        nc.vector.memset(_ap(R, 64 * G, [[G, 48], [1, 130]]), 0.0)
        nc.vector.memset(_ap(R, 64 * G + 129 * 130, [[G, 48], [1, 130]]), 0.0)
        nc.vector.memset(_ap(R, 64 * G + 128, [[G, 48], [130, 130], [1, 2]]), 0.0)
        nc.sync.dma_start(_ap(R, 112 * G, [[G, 1], [1, G]]), ones16.ap())

        # ---- conv1x1 + bias + relu into R interior ----
        psA = _Pool(tc, name="psA", bufs=4, space="PSUM")
        for j in range(32):
            ps1 = psA.tile([M, 512], F32, tag="ps1")
            nc.tensor.matmul(
                ps1[:], w1te_sb[:], x16[:, j * 512 : (j + 1) * 512],
                start=True, stop=True,
            )
            for base, coff in ((0, 1), (64 * G, 0)):
                dst = _ap(R, base + (4 * j + 1) * 130 + coff, [[G, M], [130, 4], [1, W]])
                if (2 * j + (base != 0)) % 3 < 2:
                    nc.vector.tensor_scalar(
                        dst, ps1[:], b1_sb[:], 0.0,
                        mybir.AluOpType.add, mybir.AluOpType.max,
                    )
                else:
                    nc.scalar.activation(
                        dst, ps1[:], mybir.ActivationFunctionType.Relu,
                        bias=b1_sb[:],
                    )
        psA.close()

        # ---- conv3x3 (6 paired blocks) + exp -> F ----
        psB = _Pool(tc, name="psB", bufs=4, space="PSUM")
        for j in range(32):
            ps2 = psB.tile([SK, 512], F32, tag="ps2")
            for b, (ty, tx, _pair) in enumerate(BLOCKS):
                nc.tensor.matmul(
                    ps2[:],
                    wete_sb[:, b * SK : (b + 1) * SK],
                    _ap(R, (4 * j + ty) * 130 + tx, [[G, 113], [130, 4], [1, W]]),
                    start=(b == 0), stop=(b == len(BLOCKS) - 1),
                )
            nc.scalar.activation(
                F[:, j * 512 : (j + 1) * 512], ps2[:],
                mybir.ActivationFunctionType.Exp,
            )
        psB.close()
        pr.close()

        # ---- F^T + Z in one matmul per w-column (4 per PSUM bank) ----
        SZ = SK + S2
        psF = _Pool(tc, name="psF", bufs=4, space="PSUM")
        for wb in range(32):
            pst = psF.tile([128, 4 * SZ], F32, tag="pstf")
            for w4 in range(4):
                w = wb * 4 + w4
                nc.tensor.matmul(
                    pst[:, w4 * SZ : (w4 + 1) * SZ],
                    _ap(F, w, [[HW, SK], [W, H]]),
                    idents_sb[:],
                    start=True, stop=True,
                )
            nc.vector.tensor_copy(
                _ap(fr, wb * 4, [[SK * W, 128], [W, SK], [1, 4]]),
                _ap(pst, 0, [[4 * SZ, 128], [1, SK], [SZ, 4]]),
            )
            nc.vector.tensor_copy(
                _ap(rz, wb * 4, [[S2 * W, 128], [W, S2], [1, 4]]),
                _ap(pst, SK, [[4 * SZ, 128], [1, S2], [SZ, 4]]),
            )
        psF.close()
        pf.close()

        pxa.close()

        # ---- softmax normalize: fr *= 1/Z (broadcast over k) ----
        with nc.allow_low_precision(reason="softmax weights tolerate fp16 recip"):
            nc.vector.reciprocal(rz[:], rz[:])
        fr_bc = _ap(fr, 0, [[SK * W, 128], [K2 * W, S2], [W, K2], [1, W]])
        nc.vector.tensor_mul(
            fr_bc,
            fr_bc,
            _ap(rz, 0, [[S2 * W, 128], [W, S2], [0, K2], [1, W]]),
        )

        # ---- patch-sum pools: 3-slot ring + XO0 for shifted x copies ----
        slotP = _Pool(tc, name="slots", bufs=1)
        slots = [slotP.tile([128, XF], F16, tag=f"sl{i}") for i in range(3)]
        accP = _Pool(tc, name="acc", bufs=1)
        accs = [accP.tile([128, C * W], F16, tag=f"a{s}") for s in range(S2)]
        tmpP = _Pool(tc, name="tmp", bufs=1)
        tmp = tmpP.tile([128, C * W], F16, tag="tmp")
        acc2P = _Pool(tc, name="acc2", bufs=2)

        # slot assignment per dy phase (current pair + prefetch pair alternate)
        slot_for = {-1: (slots[0], slots[1]), 1: (slots[2], XO0),
                    -2: (slots[0], slots[1]), 2: (slots[2], XO0)}
        bufs = {0: (XE0, XO0)}

        def issue_copies(dy):
            p0, p1 = max(0, -dy), 128 - max(0, dy)
            XE, XO = slot_for[dy]
            # split each body copy into 4 partition chunks on alternating
            # HWDGE queues: one dma_start lands on ONE SDMA engine (~27GB/s),
            # so chunking is what buys DMA parallelism.
            for X, eoff in ((XE, 0), (XO, 1)):
                bounds = [p0 + (p1 - p0) * i // 4 for i in range(5)]
                for c0, c1 in zip(bounds, bounds[1:]):
                    eng = nc.sync if (eoff + c0) % 2 == 0 else nc.scalar
                    eng.dma_start(
                        _ap(X, c0 * XF, [[XF, c1 - c0], [1, XF - eoff]]),
                        _ap(XE0, (c0 + dy) * XF + eoff, [[XF, c1 - c0], [1, XF - eoff]]),
                    )
            nh = abs(dy)
            p0h = 0 if dy < 0 else 128 - dy
            for X in (XE, XO):
                nc.sync.dma_start(
                    _ap(X, p0h * XF, [[XF, nh], [1, XF]]), zer16.ap()[0:nh, :]
                )
            bufs[dy] = (XE, XO)

        # ---- pixel shuffle + store: per (s, eighth) ----
        def store_s(s):
            for q in range(8):
                acc2 = acc2P.tile([128, 1024], F32, tag="acc2")
                # iteration (c4', r1, w, r2); src channel c = 8q + 4c4' + 2r1 + r2
                nc.scalar.copy(
                    _ap(acc2, 0, [[1024, 128], [512, 2], [256, 2], [1, 2], [2, W]]),
                    _ap(
                        accs[s],
                        8 * q * W,
                        [[C * W, 128], [4 * W, 2], [2 * W, 2], [W, 2], [1, W]],
                    ),
                )
                for c4 in range(2):
                    for hh in range(2):
                        dst = bass.AP(
                            tensor=out,
                            offset=(s * 16 + 2 * q + c4) * 4 * HW + hh * 64 * 512,
                            ap=[[2 * 2 * W, 64], [1, 512]],
                        )
                        eng = nc.sync if (q + c4 + hh) % 2 == 0 else nc.scalar
                        eng.dma_start(
                            dst,
                            _ap(acc2, hh * 64 * 1024 + c4 * 512, [[1024, 64], [1, 512]]),
                        )

        # ---- patch sum on VectorE ----
        DY_ORDER = [0, -1, 1, -2, 2]
        DX_ORDER = [-2, 0, 2, -1, 1]

        for di, dy in enumerate(DY_ORDER):
            if di + 1 < len(DY_ORDER):
                issue_copies(DY_ORDER[di + 1])
            XE, XO = bufs.pop(dy)
            for s in range(S2):
                for dx in DX_ORDER:
                    k = (dy + 2) * 5 + (dx + 2)
                    if dx % 2 == 0:
                        src, off = XE, 2 + dx
                    else:
                        src, off = XO, 1 + dx
                    in0 = _ap(src, off, [[XF, 128], [WP, C], [1, W]])
                    in1 = _ap(
                        fr, (s * K2 + k) * W, [[SK * W, 128], [0, C], [1, W]]
                    )
                    if dy == 0 and dx == DX_ORDER[0]:
                        dst = _ap(accs[s], 0, [[C * W, 128], [W, C], [1, W]])
                        nc.vector.tensor_mul(dst, in0, in1)
                    else:
                        dstt = _ap(tmp, 0, [[C * W, 128], [W, C], [1, W]])
                        nc.vector.tensor_mul(dstt, in0, in1)
                        nc.vector.tensor_add(accs[s][:], accs[s][:], tmp[:])
                if dy == DY_ORDER[-1]:
                    store_s(s)

        acc2P.close()
        tmpP.close()
        accP.close()
        slotP.close()
        pxo0.close()
        prz.close()
        pfr.close()
        pxe0.close()
        cp.close()
    nc.compile()
    return nc


def host_inputs(x_img, w_compress, b_compress, w_encoder, b_encoder):
    """Per-core input map for one image [C, H, W]."""
    w1te = w_compress[:, :, 0, 0].T.astype(np.float16)
    wete = np.zeros((113, 6, SK), np.float16)
    for b, (ty, tx, pair) in enumerate(BLOCKS):
        wete[:M, b, :] = w_encoder[:, :, ty, tx].T
        if pair:
            wete[64 : 64 + M, b, :] = w_encoder[:, :, ty, tx + 1].T
    wete[112, 1, :] = b_encoder
    identc = np.eye(C, dtype=np.float16)
    idents = np.zeros((SK, SK + S2), np.float16)
    idents[:, :SK] = np.eye(SK)
    for s in range(S2):
        idents[s * K2 : (s + 1) * K2, SK + s] = 1.0
    return {
        "xin": np.ascontiguousarray(x_img.reshape(C, HW)).astype(np.float32),
        "w1te": w1te,
        "b1": b_compress.reshape(M, 1).astype(np.float32),
        "wete": wete.reshape(113, 6 * SK),
        "identc": identc,
        "idents": idents,
        "ones16": np.ones((1, G), np.float16),
        "zer16": np.zeros((2, XF), np.float16),
    }


_CACHE = {}


def kernel(x, w_compress, b_compress, w_encoder, b_encoder):
    x = np.asarray(x, np.float32)
    if "nc" not in _CACHE:
        _CACHE["nc"] = build_program()
    nc = _CACHE["nc"]
    in_maps = [
        host_inputs(
            x[i],
            np.asarray(w_compress, np.float32),
            np.asarray(b_compress, np.float32),
            np.asarray(w_encoder, np.float32),
            np.asarray(b_encoder, np.float32),
        )
        for i in range(N_CORES)
    ]
    from concourse.bass_utils import run_bass_kernel_spmd

    res = run_bass_kernel_spmd(nc, in_maps, core_ids=list(range(N_CORES)))
    return np.stack(
        [res.results[i]["out"].reshape(C, 2 * H, 2 * W) for i in range(N_CORES)],
        axis=0,
    )


# revision 29
# speedup vs baseline: 1.0652x; 1.0616x over previous
"""CARAFE kernel for Trainium2 (8 NeuronCores, batch-parallel), v3.

Reference computation per image (one per core):
  R = relu(conv1x1(x, w_compress, b_compress))          [48, 128, 128]
  E = conv3x3(R, w_encoder, b_encoder, pad=1)           [100, 128, 128]
  Y = softmax over k of E.reshape(4, 25, H, W)          (s, k, h, w)
  out[s,c,h,w] = sum_k Y[s,k,h,w] * xpad[c, h+dy, w+dx] (k=(dy,dx), 5x5, pad 2)
  pixel-shuffle: out_ref[s*16 + c//4, 2h + (c//2)%2, 2w + c%2] = out[s,c,h,w]

Mapping (all 16-bit datapaths; measured E range is [-3.2, 3.3] so fp16
holds exp(E) and every intermediate comfortably):
  - conv1x1: fp16 matmuls, bias+relu fused in the PSUM->SBUF activation
    (bias as a per-partition vector).
  - conv3x3 over a zero-padded 130x130 R grid, 6 matmuls per 512-pixel
    chunk instead of 9: the R grid holds a second copy of R shifted by one
    element (rows 48-95, built by one SBUF->SBUF DMA), so taps (ty,0) and
    (ty,1) share a matmul with a 97-row contraction.  Bias via a ones row.
    exp fused into the PSUM->SBUF copy on ScalarE.
  - F^T transpose and the softmax denominator in ONE matmul per w-column:
    the moving operand is [I_100 | S] where S sums each s-group of 25, so
    PSUM gets F^T columns and Z^T columns together.  PSUM->SBUF epilogue
    copies run on VectorE (idle during the prefix); ScalarE keeps
    relu/exp/X^T epilogues.
  - X^T runs on the PE after conv3x3 (off the critical path to the
    normalize).  Its PSUM epilogue writes BOTH parity copies (XE0 and
    XO_0) so the dy=0 taps never wait on a DMA.
  - patch sum on VectorE in fp16 (2x perf mode): pixel-major layout
    [128 h-partitions, (c, w)].  dx taps are free-dim offsets; odd dx
    reads a one-element-shifted copy so every operand stays 4B-aligned.
    dy taps read partition-shifted copies of XE0 built by contiguous
    full-row SBUF->SBUF DMA into a 3-slot ring (+ the XO_0 tile), all
    prefetched one dy-phase ahead; edge partitions are zeroed (halo DMA).
  - pixel shuffle via a strided ScalarE copy (fp16->fp32) into
    (c4, r1, w, r2) order, then DMA with 2KB-contiguous runs.
"""

import sys

import numpy as np

sys.path.insert(0, "/opt/trn_rl_repo")

import concourse.bass as bass
import concourse.mybir as mybir
import concourse.tile as tile
from concourse import bacc

F32 = mybir.dt.float32
F16 = mybir.dt.float16

H = 128
W = 128
C = 64
M = 48  # compressed channels
S2 = 4  # scale_factor**2
K2 = 25  # k_up**2
SK = 100
HW = H * W
WP = 132  # padded row width in pixel-major x buffers
XF = C * WP  # 8448 free elems per partition
G = 130 * 130  # padded R grid
N_CORES = 8

# conv3x3 tap blocks: (moving ty, moving tx); rows 0-47 of the stationary
# hold tap (ty,tx), rows 48-95 hold tap (ty,tx+1) via the shifted R copy,
# row 96 rides the ones row (bias on block 1 only).
BLOCKS = [(0, 0, True), (1, 0, True), (2, 0, True), (0, 2, False), (1, 2, False), (2, 2, False)]


def _ap(t, extra_off, dims):
    """Raw AP on a tile handle `t` with free-offset `extra_off` (elements)
    and explicit [step, count] dims (dims[0] is the partition dim)."""
    base = t[:]
    return bass.AP(tensor=base.tensor, offset=base.offset + extra_off, ap=dims)


class _Pool:
    """Manually scoped tile pool."""

    def __init__(self, tc, **kw):
        self._cm = tc.tile_pool(**kw)
        self.pool = self._cm.__enter__()
        self._n = 0

    def tile(self, *a, tag=None, **kw):
        self._n += 1
        t = tag or f"t{self._n}"
        return self.pool.tile(*a, tag=t, name=t, **kw)

    def close(self):
        self._cm.__exit__(None, None, None)


def build_program():
    nc = bacc.Bacc("TRN2", target_bir_lowering=False, debug=False)

    xin = nc.dram_tensor("xin", [C, HW], F32, kind="ExternalInput")
    w1te = nc.dram_tensor("w1te", [C, M], F16, kind="ExternalInput")
    b1 = nc.dram_tensor("b1", [M, 1], F32, kind="ExternalInput")
    wete = nc.dram_tensor("wete", [113, 6 * SK], F16, kind="ExternalInput")
    identc = nc.dram_tensor("identc", [C, C], F16, kind="ExternalInput")
    idents = nc.dram_tensor("idents", [SK, SK + S2], F16, kind="ExternalInput")
    ones16 = nc.dram_tensor("ones16", [1, G], F16, kind="ExternalInput")
    zer16 = nc.dram_tensor("zer16", [2, XF], F16, kind="ExternalInput")
    out = nc.dram_tensor("out", [C, 4 * HW], F32, kind="ExternalOutput")
    x16d = nc.dram_tensor("x16d", [C, HW], F16, kind="Internal")

    with tile.TileContext(nc) as tc:
        cp = _Pool(tc, name="consts", bufs=1)
        w1te_sb = cp.tile([C, M], F16, tag="w1te")
        nc.sync.dma_start(w1te_sb[:], w1te.ap())
        b1_sb = cp.tile([M, 1], F32, tag="b1")
        nc.sync.dma_start(b1_sb[:], b1.ap())
        wete_sb = cp.tile([113, 6 * SK], F16, tag="wete")
        nc.sync.dma_start(wete_sb[:], wete.ap())
        identc_sb = cp.tile([C, C], F16, tag="identc")
        nc.sync.dma_start(identc_sb[:], identc.ap())
        idents_sb = cp.tile([SK, SK + S2], F16, tag="idents")
        nc.sync.dma_start(idents_sb[:], idents.ap())

        # persistent through the patch-sum phase
        pxe0 = _Pool(tc, name="pxe0", bufs=1)
        XE0 = pxe0.tile([128, XF], F16, tag="xe0")
        pfr = _Pool(tc, name="pfr", bufs=1)
        fr = pfr.tile([128, SK * W], F16, tag="fr")
        prz = _Pool(tc, name="prz", bufs=1)
        rz = prz.tile([128, S2 * W], F16, tag="rz")
        pxo0 = _Pool(tc, name="pxo0", bufs=1)
        XO0 = pxo0.tile([128, XF], F16, tag="xo0")
        # w-halo columns of the pixel-major x buffers (before the DMA builds)
        nc.vector.memset(_ap(XE0, 0, [[XF, 128], [WP, C], [1, 2]]), 0.0)
        nc.vector.memset(_ap(XE0, 130, [[XF, 128], [WP, C], [1, 2]]), 0.0)
        nc.vector.memset(_ap(XO0, 0, [[XF, 128], [WP, C], [1, 1]]), 0.0)
        nc.vector.memset(_ap(XO0, 129, [[XF, 128], [WP, C], [1, 3]]), 0.0)

        # ---- load x first (cast fp32->fp16 during DMA, 4 parallel engines) ----
        pxa = _Pool(tc, name="pxa", bufs=1)
        x16 = pxa.tile([C, HW], F16, tag="x16")
        for c in range(0, C, 8):
            nc.gpsimd.dma_start(x16[c : c + 8, :], xin.ap()[c : c + 8, :])

        # stage x16 to DRAM, then build the pixel-major XE0/XO0 buffers with
        # strided DMAs (bypasses the PE transpose entirely; runs on otherwise
        # idle DMA engines in parallel with the conv chain)
        for i in range(4):
            c0, c1 = C * i // 4, C * (i + 1) // 4
            nc.gpsimd.dma_start(x16d.ap()[c0:c1, :], x16[c0:c1, :])
        for i in range(8):
            p0, p1 = 128 * i // 8, 128 * (i + 1) // 8
            nc.gpsimd.dma_start(
                _ap(XE0, p0 * XF + 2, [[XF, p1 - p0], [WP, C], [1, W]]),
                bass.AP(tensor=x16d, offset=p0 * W,
                        ap=[[W, p1 - p0], [HW, C], [1, W]]),
            )
            nc.gpsimd.dma_start(
                _ap(XO0, p0 * XF + 1, [[XF, p1 - p0], [WP, C], [1, W]]),
                bass.AP(tensor=x16d, offset=p0 * W,
                        ap=[[W, p1 - p0], [HW, C], [1, W]]),
            )

        pf = _Pool(tc, name="pf", bufs=1)
        F = pf.tile([SK, HW], F16, tag="F")

        # ---- R grid (with shifted duplicate rows 48-95, ones row 96) ----
        pr = _Pool(tc, name="pr", bufs=1)
        R = pr.tile([113, G], F16, tag="R")
        # zero only the padding cells (grid rows 0/129, cols 0/129 for rows
        # 0-47; the shifted duplicate rows inherit them via the dup copy)
        nc.vector.memset(_ap(R, 0, [[G, M], [1, 130]]), 0.0)
        nc.vector.memset(_ap(R, 129 * 130, [[G, M], [1, 130]]), 0.0)
        nc.vector.memset(_ap(R, 130, [[G, M], [130, 128], [1, 1]]), 0.0)
        nc.vector.memset(_ap(R, 130 + 129, [[G, M], [130, 128], [1, 1]]), 0.0)
        nc.vector.memset(_ap(R, 32 * G, [[G, 32], [1, G]]), 0.0)
        nc.vector.memset(_ap(R, 64 * G, [[G, 48], [1, 130]]), 0.0)
        nc.vector.memset(_ap(R, 64 * G + 129 * 130, [[G, 48], [1, 130]]), 0.0)
        nc.vector.memset(_ap(R, 64 * G + 128, [[G, 48], [130, 130], [1, 2]]), 0.0)
        nc.sync.dma_start(_ap(R, 112 * G, [[G, 1], [1, G]]), ones16.ap())

        # ---- conv1x1 + bias + relu into R interior ----
        psA = _Pool(tc, name="psA", bufs=4, space="PSUM")
        for j in range(32):
            ps1 = psA.tile([M, 512], F32, tag="ps1")
            nc.tensor.matmul(
                ps1[:], w1te_sb[:], x16[:, j * 512 : (j + 1) * 512],
                start=True, stop=True,
            )
            for base, coff in ((0, 1), (64 * G, 0)):
                dst = _ap(R, base + (4 * j + 1) * 130 + coff, [[G, M], [130, 4], [1, W]])
                if (2 * j + (base != 0)) % 3 < 2:
                    nc.vector.tensor_scalar(
                        dst, ps1[:], b1_sb[:], 0.0,
                        mybir.AluOpType.add, mybir.AluOpType.max,
                    )
                else:
                    nc.scalar.activation(
                        dst, ps1[:], mybir.ActivationFunctionType.Relu,
                        bias=b1_sb[:],
                    )
        psA.close()

        # ---- conv3x3 (6 paired blocks) + exp -> F ----
        psB = _Pool(tc, name="psB", bufs=4, space="PSUM")
        for j in range(32):
            ps2 = psB.tile([SK, 512], F32, tag="ps2")
            for b, (ty, tx, _pair) in enumerate(BLOCKS):
                nc.tensor.matmul(
                    ps2[:],
                    wete_sb[:, b * SK : (b + 1) * SK],
                    _ap(R, (4 * j + ty) * 130 + tx, [[G, 113], [130, 4], [1, W]]),
                    start=(b == 0), stop=(b == len(BLOCKS) - 1),
                )
            nc.scalar.activation(
                F[:, j * 512 : (j + 1) * 512], ps2[:],
                mybir.ActivationFunctionType.Exp,
            )
        psB.close()
        pr.close()

        # ---- F^T + Z in one matmul per w-column (4 per PSUM bank) ----
        SZ = SK + S2
        psF = _Pool(tc, name="psF", bufs=4, space="PSUM")
        for wb in range(32):
            pst = psF.tile([128, 4 * SZ], F32, tag="pstf")
            for w4 in range(4):
                w = wb * 4 + w4
                nc.tensor.matmul(
                    pst[:, w4 * SZ : (w4 + 1) * SZ],
                    _ap(F, w, [[HW, SK], [W, H]]),
                    idents_sb[:],
                    start=True, stop=True,
                )
            nc.vector.tensor_copy(
                _ap(fr, wb * 4, [[SK * W, 128], [W, SK], [1, 4]]),
                _ap(pst, 0, [[4 * SZ, 128], [1, SK], [SZ, 4]]),
            )
            nc.vector.tensor_copy(
                _ap(rz, wb * 4, [[S2 * W, 128], [W, S2], [1, 4]]),
                _ap(pst, SK, [[4 * SZ, 128], [1, S2], [SZ, 4]]),
            )
        psF.close()
        pf.close()

        pxa.close()

        # ---- softmax normalize: fr *= 1/Z (broadcast over k) ----
        with nc.allow_low_precision(reason="softmax weights tolerate fp16 recip"):
            nc.vector.reciprocal(rz[:], rz[:])
        fr_bc = _ap(fr, 0, [[SK * W, 128], [K2 * W, S2], [W, K2], [1, W]])
        nc.vector.tensor_mul(
            fr_bc,
            fr_bc,
            _ap(rz, 0, [[S2 * W, 128], [W, S2], [0, K2], [1, W]]),
        )

        # ---- patch-sum pools: 3-slot ring + XO0 for shifted x copies ----
        slotP = _Pool(tc, name="slots", bufs=1)
        slots = [slotP.tile([128, XF], F16, tag=f"sl{i}") for i in range(3)]
        accP = _Pool(tc, name="acc", bufs=1)
        accs = [accP.tile([128, C * W], F16, tag=f"a{s}") for s in range(S2)]
        tmpP = _Pool(tc, name="tmp", bufs=1)
        tmp = tmpP.tile([128, C * W], F16, tag="tmp")
        acc2P = _Pool(tc, name="acc2", bufs=2)

        # slot assignment per dy phase (current pair + prefetch pair alternate)
        slot_for = {-1: (slots[0], slots[1]), 1: (slots[2], XO0),
                    -2: (slots[0], slots[1]), 2: (slots[2], XO0)}
        bufs = {0: (XE0, XO0)}

        def issue_copies(dy):
            p0, p1 = max(0, -dy), 128 - max(0, dy)
            XE, XO = slot_for[dy]
            # split each body copy into 4 partition chunks on alternating
            # HWDGE queues: one dma_start lands on ONE SDMA engine (~27GB/s),
            # so chunking is what buys DMA parallelism.
            for X, eoff in ((XE, 0), (XO, 1)):
                bounds = [p0 + (p1 - p0) * i // 4 for i in range(5)]
                for c0, c1 in zip(bounds, bounds[1:]):
                    eng = nc.sync if (eoff + c0) % 2 == 0 else nc.scalar
                    eng.dma_start(
                        _ap(X, c0 * XF, [[XF, c1 - c0], [1, XF - eoff]]),
                        _ap(XE0, (c0 + dy) * XF + eoff, [[XF, c1 - c0], [1, XF - eoff]]),
                    )
            nh = abs(dy)
            p0h = 0 if dy < 0 else 128 - dy
            for X in (XE, XO):
                nc.sync.dma_start(
                    _ap(X, p0h * XF, [[XF, nh], [1, XF]]), zer16.ap()[0:nh, :]
                )
            bufs[dy] = (XE, XO)

        # ---- pixel shuffle + store: per (s, eighth) ----
        def store_s(s):
            for q in range(8):
                acc2 = acc2P.tile([128, 1024], F32, tag="acc2")
                # iteration (c4', r1, w, r2); src channel c = 8q + 4c4' + 2r1 + r2
                nc.scalar.copy(
                    _ap(acc2, 0, [[1024, 128], [512, 2], [256, 2], [1, 2], [2, W]]),
                    _ap(
                        accs[s],
                        8 * q * W,
                        [[C * W, 128], [4 * W, 2], [2 * W, 2], [W, 2], [1, W]],
                    ),
                )
                for c4 in range(2):
                    for hh in range(2):
                        dst = bass.AP(
                            tensor=out,
                            offset=(s * 16 + 2 * q + c4) * 4 * HW + hh * 64 * 512,
                            ap=[[2 * 2 * W, 64], [1, 512]],
                        )
                        eng = nc.sync if (q + c4 + hh) % 2 == 0 else nc.scalar
                        eng.dma_start(
                            dst,
                            _ap(acc2, hh * 64 * 1024 + c4 * 512, [[1024, 64], [1, 512]]),
                        )

        # ---- patch sum on VectorE ----
        DY_ORDER = [0, -1, 1, -2, 2]
        DX_ORDER = [-2, 0, 2, -1, 1]

        for di, dy in enumerate(DY_ORDER):
            if di + 1 < len(DY_ORDER):
                issue_copies(DY_ORDER[di + 1])
            XE, XO = bufs.pop(dy)
            for s in range(S2):
                for dx in DX_ORDER:
                    k = (dy + 2) * 5 + (dx + 2)
                    if dx % 2 == 0:
                        src, off = XE, 2 + dx
                    else:
                        src, off = XO, 1 + dx
                    in0 = _ap(src, off, [[XF, 128], [WP, C], [1, W]])
                    in1 = _ap(
                        fr, (s * K2 + k) * W, [[SK * W, 128], [0, C], [1, W]]
                    )
                    if dy == 0 and dx == DX_ORDER[0]:
                        dst = _ap(accs[s], 0, [[C * W, 128], [W, C], [1, W]])
                        nc.vector.tensor_mul(dst, in0, in1)
                    else:
                        dstt = _ap(tmp, 0, [[C * W, 128], [W, C], [1, W]])
                        nc.vector.tensor_mul(dstt, in0, in1)
                        nc.vector.tensor_add(accs[s][:], accs[s][:], tmp[:])
                if dy == DY_ORDER[-1]:
                    store_s(s)

        acc2P.close()
        tmpP.close()
        accP.close()
        slotP.close()
        pxo0.close()
        prz.close()
        pfr.close()
        pxe0.close()
        cp.close()
    nc.compile()
    return nc


def host_inputs(x_img, w_compress, b_compress, w_encoder, b_encoder):
    """Per-core input map for one image [C, H, W]."""
    w1te = w_compress[:, :, 0, 0].T.astype(np.float16)
    wete = np.zeros((113, 6, SK), np.float16)
    for b, (ty, tx, pair) in enumerate(BLOCKS):
        wete[:M, b, :] = w_encoder[:, :, ty, tx].T
        if pair:
            wete[64 : 64 + M, b, :] = w_encoder[:, :, ty, tx + 1].T
    wete[112, 1, :] = b_encoder
    identc = np.eye(C, dtype=np.float16)
    idents = np.zeros((SK, SK + S2), np.float16)
    idents[:, :SK] = np.eye(SK)
    for s in range(S2):
        idents[s * K2 : (s + 1) * K2, SK + s] = 1.0
    return {
        "xin": np.ascontiguousarray(x_img.reshape(C, HW)).astype(np.float32),
        "w1te": w1te,
        "b1": b_compress.reshape(M, 1).astype(np.float32),
        "wete": wete.reshape(113, 6 * SK),
        "identc": identc,
        "idents": idents,
        "ones16": np.ones((1, G), np.float16),
        "zer16": np.zeros((2, XF), np.float16),
    }


_CACHE = {}


def kernel(x, w_compress, b_compress, w_encoder, b_encoder):
    x = np.asarray(x, np.float32)
    if "nc" not in _CACHE:
        _CACHE["nc"] = build_program()
    nc = _CACHE["nc"]
    in_maps = [
        host_inputs(
            x[i],
            np.asarray(w_compress, np.float32),
            np.asarray(b_compress, np.float32),
            np.asarray(w_encoder, np.float32),
            np.asarray(b_encoder, np.float32),
        )
        for i in range(N_CORES)
    ]
    from concourse.bass_utils import run_bass_kernel_spmd

    res = run_bass_kernel_spmd(nc, in_maps, core_ids=list(range(N_CORES)))
    return np.stack(
        [res.results[i]["out"].reshape(C, 2 * H, 2 * W) for i in range(N_CORES)],
        axis=0,
    )
